# revision 45
# baseline (speedup 1.0000x reference)
"""Trainium2 kernel for the FEM kinematic (strain) layer.

Reference computation:
    disp = inputs[:, elem_nodes]                      # [B, E, 8, 2]
    dd   = einsum('egkl,bekn->begnl', shpdx, disp)    # [B, E, 9, 2, 2]
    out  = stack([dd[...,0,0], dd[...,1,1],
                  0.5*(dd[...,0,1] + dd[...,1,0])])   # [B, E*9, 3]

Sharding: elements split across 8 NeuronCores.  The host resolves the
element->node indirection and ships per-core element-major blocks in
bf16: the two shpdx l-planes (S0, S1), their sum A, and displacement
planes u, v, w=u+v laid out (b, c, k) per partition.  The device
computes, per element and gauss point,
    e_xx = sum_k S0*u,  e_yy = sum_k S1*v,
    e_xy = 0.5*(sum_k A*w - e_xx - e_yy)
with three batched bf16 multiplies (DVE 2x packed mode), segmented
k-sums as packed add-trees (8 -> 4 -> 2 -> 1) split between DVE (xx/yy)
and Pool (xy path + subtracts), and the 0.5-scale on Activation.
30 full chunks of 2048 elements plus one 1152-element tail chunk
(92 pad elements instead of 988).  Output is written bf16 and widened
on the host.
"""

import sys
import numpy as np

sys.path.insert(0, "/opt/trn_rl_repo")

import ml_dtypes

import concourse.bass as bass
import concourse.bacc as bacc
import concourse.mybir as mybir
import concourse.tile as tile
from concourse.bass_utils import run_bass_kernel_spmd

BF16 = ml_dtypes.bfloat16

B = 4
N_NODES = 1_000_000
N_ELEM = 500_000
N_GP = 9
N_EN = 8
N_CORES = 8

E_CORE = N_ELEM // N_CORES            # 62500 elements per core
P = 128                               # SBUF partitions
C = 16                                # elements per partition, full chunk
# small chunk first (short pipeline fill), 30 full chunks after; the 92
# pad elements ride in the last full chunk
_C_LIST = [9] + [16] * 30
SEGS = []
_e = 0
for _c in _C_LIST:
    SEGS.append((_e, _c))
    _e += P * _c
E_PAD = _e                            # 62592 (92 pad elements)

_compiled = None


def _build_program():
    nc = bacc.Bacc("TRN2", target_bir_lowering=False, debug=False)
    bf = mybir.dt.bfloat16

    # flat 1D element-major streams
    s0_d = nc.dram_tensor("s0", [E_PAD * 72], bf, kind="ExternalInput").ap()
    s1_d = nc.dram_tensor("s1", [E_PAD * 72], bf, kind="ExternalInput").ap()
    a_d = nc.dram_tensor("a", [E_PAD * 72], bf, kind="ExternalInput").ap()
    # displacement planes, flat per-chunk (p, b, c, k) blocks
    du_d = nc.dram_tensor("du", [E_PAD * B * 8], bf, kind="ExternalInput").ap()
    dv_d = nc.dram_tensor("dv", [E_PAD * B * 8], bf, kind="ExternalInput").ap()
    dw_d = nc.dram_tensor("dw", [E_PAD * B * 8], bf, kind="ExternalInput").ap()
    # [B, E_PAD*27] bf16 (host reshapes to [B, E_PAD*9, 3])
    o_d = nc.dram_tensor("out", [B, E_PAD * 27], bf, kind="ExternalOutput").ap()

    add = mybir.AluOpType.add
    sub = mybir.AluOpType.subtract
    mult = mybir.AluOpType.mult

    with tile.TileContext(nc) as tc:
        with (
            tc.tile_pool(name="io", bufs=3) as io_pool,
            tc.tile_pool(name="tmp", bufs=3) as tmp_pool,
        ):
            prev = None
            for i, (e0, c) in enumerate(SEGS):
                S0 = io_pool.tile([P, C * 72], bf, tag="S0")
                S1 = io_pool.tile([P, C * 72], bf, tag="S1")
                A = io_pool.tile([P, C * 72], bf, tag="A")
                Du = io_pool.tile([P, B * C * 8], bf, tag="Du")
                Dv = io_pool.tile([P, B * C * 8], bf, tag="Dv")
                Dw = io_pool.tile([P, B * C * 8], bf, tag="Dw")

                def sload(t, src):
                    nc.sync.dma_start(
                        out=t[:, :c * 72],
                        in_=src[e0 * 72:(e0 + P * c) * 72].rearrange(
                            "(p x) -> p x", p=P
                        ),
                    )

                def dload(t, src):
                    nc.sync.dma_start(
                        out=t[:, :B * c * 8],
                        in_=src[e0 * B * 8:(e0 + P * c) * B * 8].rearrange(
                            "(p x) -> p x", p=P
                        ),
                    )

                sload(A, a_d)
                dload(Dw, dw_d)
                sload(S0, s0_d)
                dload(Du, du_d)
                sload(S1, s1_d)
                dload(Dv, dv_d)

                O = io_pool.tile([P, B * C * 27], bf, tag="O")
                # (bc, t, g) view of the (b, c, g, t) staging layout
                Oxy = O[:, :B * c * 27].rearrange(
                    "p (b c g t) -> p (b c) t g", b=B, c=c, g=9
                )

                def splane(t):
                    r = t[:, :c * 72].rearrange("p (c g k) -> p c g k", c=c, g=9)
                    return r[:, None, :, :, :].to_broadcast([P, B, c, 9, 8])

                def dplane(t):
                    r = t[:, :B * c * 8].rearrange("p (b c k) -> p b c k", b=B, c=c)
                    return r[:, :, :, None, :].to_broadcast([P, B, c, 9, 8])

                # ---- products ------------------------------------------
                # T01[l, b, c, g, k]: l=0 -> S0*u (xx), l=1 -> S1*v (yy)
                T01 = tmp_pool.tile([P, 2 * B * C * 72], bf, tag="T01")
                T01v = T01[:, :2 * B * c * 72].rearrange(
                    "p (l b c g k) -> p l b c g k", l=2, b=B, c=c, g=9
                )
                T2 = tmp_pool.tile([P, B * C * 72], bf, tag="T2")
                T2v = T2[:, :B * c * 72].rearrange(
                    "p (b c g k) -> p b c g k", b=B, c=c, g=9
                )
                nc.vector.tensor_tensor(
                    out=T2v, in0=splane(A), in1=dplane(Dw), op=mult
                )
                nc.vector.tensor_tensor(
                    out=T01v[:, 0], in0=splane(S0), in1=dplane(Du), op=mult
                )
                nc.vector.tensor_tensor(
                    out=T01v[:, 1], in0=splane(S1), in1=dplane(Dv), op=mult
                )

                # xy tree: R4 on Pool, R2 mostly on DVE, s3xy on Pool
                T2f = T2[:, :B * c * 72].rearrange(
                    "p (bc g k) -> p bc g k", g=9, k=8
                )
                R4 = tmp_pool.tile([P, B * C * 36], bf, tag="R4")
                R4v = R4[:, :B * c * 36].rearrange(
                    "p (bc g k) -> p bc g k", g=9, k=4
                )
                nc.gpsimd.tensor_tensor(
                    out=R4v, in0=T2f[:, :, :, 0:4], in1=T2f[:, :, :, 4:8],
                    op=add,
                )

                # ---- k-sum add-trees ------------------------------------
                # xx+yy tree on DVE, fused over (l b c): [*, g, 8]->4->2->O
                T01f = T01[:, :2 * B * c * 72].rearrange(
                    "p (q g k) -> p q g k", g=9, k=8
                )
                Q4 = tmp_pool.tile([P, 2 * B * C * 36], bf, tag="Q4")
                Q4v = Q4[:, :2 * B * c * 36].rearrange(
                    "p (q g k) -> p q g k", g=9, k=4
                )
                nc.vector.tensor_tensor(
                    out=Q4v, in0=T01f[:, :, :, 0:4], in1=T01f[:, :, :, 4:8],
                    op=add,
                )
                Q2 = tmp_pool.tile([P, 2 * B * C * 18], bf, tag="Q2")
                Q2v = Q2[:, :2 * B * c * 18].rearrange(
                    "p (q g k) -> p q g k", g=9, k=2
                )
                nc.vector.tensor_tensor(
                    out=Q2v, in0=Q4v[:, :, :, 0:2], in1=Q4v[:, :, :, 2:4],
                    op=add,
                )
                # stage3 fused: dims (l, bc, g) -> O[t=l]; on Pool for a
                # subset of chunks to balance engine load
                Q2t = Q2[:, :2 * B * c * 18].rearrange(
                    "p (l bc g k) -> p l bc g k", l=2, g=9, k=2
                )
                nc.gpsimd.tensor_tensor(
                    out=Oxy[:, :, 0:2, :].rearrange("p bc t g -> p t bc g"),
                    in0=Q2t[:, :, :, :, 0], in1=Q2t[:, :, :, :, 1],
                    op=add,
                )

                R2 = tmp_pool.tile([P, B * C * 18], bf, tag="R2")
                R2v = R2[:, :B * c * 18].rearrange(
                    "p (bc g k) -> p bc g k", g=9, k=2
                )
                r2eng = nc.vector
                r2eng.tensor_tensor(
                    out=R2v, in0=R4v[:, :, :, 0:2], in1=R4v[:, :, :, 2:4], op=add
                )
                # previous chunk's tail ops keep Pool busy while R2 lands
                if prev is not None:
                    _finish(nc, o_d, prev)
                X2 = tmp_pool.tile([P, B * C * 9], bf, tag="X2")
                X2v = X2[:, :B * c * 9].rearrange("p (bc g) -> p bc g", g=9)
                nc.gpsimd.tensor_tensor(
                    out=X2v, in0=R2v[:, :, :, 0], in1=R2v[:, :, :, 1], op=add
                )
                prev = (O, Oxy, X2v, e0, c)
            _finish(nc, o_d, prev)

    nc.compile()
    return nc


def _finish(nc, o_d, handles):
    O, Oxy, X2v, e0, c = handles
    sub = mybir.AluOpType.subtract
    nc.gpsimd.tensor_tensor(out=X2v, in0=X2v, in1=Oxy[:, :, 0, :], op=sub)
    nc.gpsimd.tensor_tensor(out=X2v, in0=X2v, in1=Oxy[:, :, 1, :], op=sub)
    # e_xy = 0.5 * X2  (Activation)
    nc.scalar.activation(
        out=Oxy[:, :, 2, :], in_=X2v,
        func=mybir.ActivationFunctionType.Copy, scale=0.5,
    )
    for b in range(B):
        nc.sync.dma_start(
            out=o_d[b, e0 * 27:(e0 + P * c) * 27].rearrange("(p x) -> p x", p=P),
            in_=O[:, b * c * 27:(b + 1) * c * 27],
        )


def _get_program():
    global _compiled
    if _compiled is None:
        _compiled = _build_program()
    return _compiled


def kernel(inputs, shpdx, elem_nodes, _want_trace=False):
    nc = _get_program()

    in_maps = []
    for core in range(N_CORES):
        sl = slice(core * E_CORE, (core + 1) * E_CORE)
        sp = shpdx[sl]                                        # [E, 9, 8, 2] f32

        def pad72(x):
            out = np.zeros((E_PAD, 72), BF16)
            out[:E_CORE] = x.reshape(E_CORE, 72).astype(BF16)
            return out.reshape(E_PAD * 72)

        s0 = pad72(np.ascontiguousarray(sp[..., 0]))          # (g, k)
        s1 = pad72(np.ascontiguousarray(sp[..., 1]))
        a = pad72(sp.sum(axis=3))

        en = elem_nodes[sl]                                   # [E, 8]
        disp = inputs[:, en]                                  # [B, E, 8, 2]

        # displacement planes: per chunk, (p, b, c, k) blocks, flat
        def dplane(x):                                        # x: [B, E, 8] f32
            xp = np.zeros((B, E_PAD, 8), np.float32)
            xp[:, :E_CORE] = x
            parts = []
            for e0, c in SEGS:
                blk = xp[:, e0:e0 + P * c]                    # [B, P*c, 8]
                blk = blk.reshape(B, P, c, 8).transpose(1, 0, 2, 3)
                parts.append(np.ascontiguousarray(blk).reshape(-1))
            return np.concatenate(parts).astype(BF16)

        u = disp[..., 0]
        v = disp[..., 1]
        in_maps.append({
            "s0": s0, "s1": s1, "a": a,
            "du": dplane(u), "dv": dplane(v), "dw": dplane(u + v),
        })

    core_ids = list(range(N_CORES))
    res = run_bass_kernel_spmd(nc, in_maps, core_ids, trace=_want_trace)

    outs = []
    for core in range(N_CORES):
        o = res.results[core]["out"].reshape(B, E_PAD * 9, 3) # bf16
        outs.append(o[:, :E_CORE * 9, :].astype(np.float32))
    full = np.concatenate(outs, axis=1)                       # [B, N_ELEM*9, 3]
    if _want_trace:
        return full, res
    return full


# revision 46
# speedup vs baseline: 1.0032x; 1.0032x over previous
"""Trainium2 kernel for the FEM kinematic (strain) layer.

Reference computation:
    disp = inputs[:, elem_nodes]                      # [B, E, 8, 2]
    dd   = einsum('egkl,bekn->begnl', shpdx, disp)    # [B, E, 9, 2, 2]
    out  = stack([dd[...,0,0], dd[...,1,1],
                  0.5*(dd[...,0,1] + dd[...,1,0])])   # [B, E*9, 3]

Sharding: elements split across 8 NeuronCores.  The host resolves the
element->node indirection and ships per-core element-major blocks in
bf16: the two shpdx l-planes (S0, S1), their sum A, and displacement
planes u, v, w=u+v laid out (b, c, k) per partition.  The device
computes, per element and gauss point,
    e_xx = sum_k S0*u,  e_yy = sum_k S1*v,
    e_xy = 0.5*(sum_k A*w - e_xx - e_yy)
with three batched bf16 multiplies (DVE 2x packed mode), segmented
k-sums as packed add-trees (8 -> 4 -> 2 -> 1) split between DVE (xx/yy)
and Pool (xy path + subtracts), and the 0.5-scale on Activation.
30 full chunks of 2048 elements plus one 1152-element tail chunk
(92 pad elements instead of 988).  Output is written bf16 and widened
on the host.
"""

import sys
import numpy as np

sys.path.insert(0, "/opt/trn_rl_repo")

import ml_dtypes

import concourse.bass as bass
import concourse.bacc as bacc
import concourse.mybir as mybir
import concourse.tile as tile
from concourse.bass_utils import run_bass_kernel_spmd

BF16 = ml_dtypes.bfloat16

B = 4
N_NODES = 1_000_000
N_ELEM = 500_000
N_GP = 9
N_EN = 8
N_CORES = 8

E_CORE = N_ELEM // N_CORES            # 62500 elements per core
P = 128                               # SBUF partitions
C = 16                                # elements per partition, full chunk
# small chunk first (short pipeline fill), 30 full chunks after; the 92
# pad elements ride in the last full chunk
_C_LIST = [9] + [16] * 30
SEGS = []
_e = 0
for _c in _C_LIST:
    SEGS.append((_e, _c))
    _e += P * _c
E_PAD = _e                            # 62592 (92 pad elements)

_compiled = None


def _build_program():
    nc = bacc.Bacc("TRN2", target_bir_lowering=False, debug=False)
    bf = mybir.dt.bfloat16

    # flat 1D element-major streams
    s0_d = nc.dram_tensor("s0", [E_PAD * 72], bf, kind="ExternalInput").ap()
    s1_d = nc.dram_tensor("s1", [E_PAD * 72], bf, kind="ExternalInput").ap()
    a_d = nc.dram_tensor("a", [E_PAD * 72], bf, kind="ExternalInput").ap()
    # displacement planes, flat per-chunk (p, b, c, k) blocks
    du_d = nc.dram_tensor("du", [E_PAD * B * 8], bf, kind="ExternalInput").ap()
    dv_d = nc.dram_tensor("dv", [E_PAD * B * 8], bf, kind="ExternalInput").ap()
    dw_d = nc.dram_tensor("dw", [E_PAD * B * 8], bf, kind="ExternalInput").ap()
    # [B, E_PAD*27] bf16 (host reshapes to [B, E_PAD*9, 3])
    o_d = nc.dram_tensor("out", [B, E_PAD * 27], bf, kind="ExternalOutput").ap()

    add = mybir.AluOpType.add
    sub = mybir.AluOpType.subtract
    mult = mybir.AluOpType.mult

    with tile.TileContext(nc) as tc:
        with (
            tc.tile_pool(name="io", bufs=3) as io_pool,
            tc.tile_pool(name="tmp", bufs=3) as tmp_pool,
        ):
            prev = None
            for i, (e0, c) in enumerate(SEGS):
                S0 = io_pool.tile([P, C * 72], bf, tag="S0")
                S1 = io_pool.tile([P, C * 72], bf, tag="S1")
                A = io_pool.tile([P, C * 72], bf, tag="A")
                Du = io_pool.tile([P, B * C * 8], bf, tag="Du")
                Dv = io_pool.tile([P, B * C * 8], bf, tag="Dv")
                Dw = io_pool.tile([P, B * C * 8], bf, tag="Dw")

                def sload(t, src):
                    nc.sync.dma_start(
                        out=t[:, :c * 72],
                        in_=src[e0 * 72:(e0 + P * c) * 72].rearrange(
                            "(p x) -> p x", p=P
                        ),
                    )

                def dload(t, src):
                    nc.sync.dma_start(
                        out=t[:, :B * c * 8],
                        in_=src[e0 * B * 8:(e0 + P * c) * B * 8].rearrange(
                            "(p x) -> p x", p=P
                        ),
                    )

                sload(A, a_d)
                dload(Dw, dw_d)
                sload(S0, s0_d)
                dload(Du, du_d)
                sload(S1, s1_d)
                dload(Dv, dv_d)

                O = io_pool.tile([P, B * C * 27], bf, tag="O")
                # (bc, t, g) view of the (b, c, g, t) staging layout
                Oxy = O[:, :B * c * 27].rearrange(
                    "p (b c g t) -> p (b c) t g", b=B, c=c, g=9
                )

                def splane(t):
                    r = t[:, :c * 72].rearrange("p (c g k) -> p c g k", c=c, g=9)
                    return r[:, None, :, :, :].to_broadcast([P, B, c, 9, 8])

                def dplane(t):
                    r = t[:, :B * c * 8].rearrange("p (b c k) -> p b c k", b=B, c=c)
                    return r[:, :, :, None, :].to_broadcast([P, B, c, 9, 8])

                # ---- products ------------------------------------------
                # T01[l, b, c, g, k]: l=0 -> S0*u (xx), l=1 -> S1*v (yy)
                T01 = tmp_pool.tile([P, 2 * B * C * 72], bf, tag="T01")
                T01v = T01[:, :2 * B * c * 72].rearrange(
                    "p (l b c g k) -> p l b c g k", l=2, b=B, c=c, g=9
                )
                T2 = tmp_pool.tile([P, B * C * 72], bf, tag="T2")
                T2v = T2[:, :B * c * 72].rearrange(
                    "p (b c g k) -> p b c g k", b=B, c=c, g=9
                )
                nc.vector.tensor_tensor(
                    out=T2v, in0=splane(A), in1=dplane(Dw), op=mult
                )
                nc.vector.tensor_tensor(
                    out=T01v[:, 0], in0=splane(S0), in1=dplane(Du), op=mult
                )
                nc.vector.tensor_tensor(
                    out=T01v[:, 1], in0=splane(S1), in1=dplane(Dv), op=mult
                )

                # xy tree: R4 on Pool, R2 mostly on DVE, s3xy on Pool
                T2f = T2[:, :B * c * 72].rearrange(
                    "p (bc g k) -> p bc g k", g=9, k=8
                )
                R4 = tmp_pool.tile([P, B * C * 36], bf, tag="R4")
                R4v = R4[:, :B * c * 36].rearrange(
                    "p (bc g k) -> p bc g k", g=9, k=4
                )
                nc.gpsimd.tensor_tensor(
                    out=R4v, in0=T2f[:, :, :, 0:4], in1=T2f[:, :, :, 4:8],
                    op=add,
                )

                # ---- k-sum add-trees ------------------------------------
                # xx+yy tree on DVE, fused over (l b c): [*, g, 8]->4->2->O
                T01f = T01[:, :2 * B * c * 72].rearrange(
                    "p (q g k) -> p q g k", g=9, k=8
                )
                Q4 = tmp_pool.tile([P, 2 * B * C * 36], bf, tag="Q4")
                Q4v = Q4[:, :2 * B * c * 36].rearrange(
                    "p (q g k) -> p q g k", g=9, k=4
                )
                nc.vector.tensor_tensor(
                    out=Q4v, in0=T01f[:, :, :, 0:4], in1=T01f[:, :, :, 4:8],
                    op=add,
                )
                Q2 = tmp_pool.tile([P, 2 * B * C * 18], bf, tag="Q2")
                Q2v = Q2[:, :2 * B * c * 18].rearrange(
                    "p (q g k) -> p q g k", g=9, k=2
                )
                nc.vector.tensor_tensor(
                    out=Q2v, in0=Q4v[:, :, :, 0:2], in1=Q4v[:, :, :, 2:4],
                    op=add,
                )
                # stage3 fused: dims (l, bc, g) -> O[t=l]; on Pool for a
                # subset of chunks to balance engine load
                Q2t = Q2[:, :2 * B * c * 18].rearrange(
                    "p (l bc g k) -> p l bc g k", l=2, g=9, k=2
                )
                s3eng = nc.vector if i == len(SEGS) - 1 else nc.gpsimd
                s3eng.tensor_tensor(
                    out=Oxy[:, :, 0:2, :].rearrange("p bc t g -> p t bc g"),
                    in0=Q2t[:, :, :, :, 0], in1=Q2t[:, :, :, :, 1],
                    op=add,
                )

                R2 = tmp_pool.tile([P, B * C * 18], bf, tag="R2")
                R2v = R2[:, :B * c * 18].rearrange(
                    "p (bc g k) -> p bc g k", g=9, k=2
                )
                r2eng = nc.vector
                r2eng.tensor_tensor(
                    out=R2v, in0=R4v[:, :, :, 0:2], in1=R4v[:, :, :, 2:4], op=add
                )
                # previous chunk's tail ops keep Pool busy while R2 lands
                if prev is not None:
                    _finish(nc, o_d, prev)
                X2 = tmp_pool.tile([P, B * C * 9], bf, tag="X2")
                X2v = X2[:, :B * c * 9].rearrange("p (bc g) -> p bc g", g=9)
                nc.gpsimd.tensor_tensor(
                    out=X2v, in0=R2v[:, :, :, 0], in1=R2v[:, :, :, 1], op=add
                )
                prev = (O, Oxy, X2v, e0, c)
            _finish(nc, o_d, prev)

    nc.compile()
    return nc


def _finish(nc, o_d, handles):
    O, Oxy, X2v, e0, c = handles
    sub = mybir.AluOpType.subtract
    nc.gpsimd.tensor_tensor(out=X2v, in0=X2v, in1=Oxy[:, :, 0, :], op=sub)
    nc.gpsimd.tensor_tensor(out=X2v, in0=X2v, in1=Oxy[:, :, 1, :], op=sub)
    # e_xy = 0.5 * X2  (Activation)
    nc.scalar.activation(
        out=Oxy[:, :, 2, :], in_=X2v,
        func=mybir.ActivationFunctionType.Copy, scale=0.5,
    )
    for b in range(B):
        nc.sync.dma_start(
            out=o_d[b, e0 * 27:(e0 + P * c) * 27].rearrange("(p x) -> p x", p=P),
            in_=O[:, b * c * 27:(b + 1) * c * 27],
        )


def _get_program():
    global _compiled
    if _compiled is None:
        _compiled = _build_program()
    return _compiled


def kernel(inputs, shpdx, elem_nodes, _want_trace=False):
    nc = _get_program()

    in_maps = []
    for core in range(N_CORES):
        sl = slice(core * E_CORE, (core + 1) * E_CORE)
        sp = shpdx[sl]                                        # [E, 9, 8, 2] f32

        def pad72(x):
            out = np.zeros((E_PAD, 72), BF16)
            out[:E_CORE] = x.reshape(E_CORE, 72).astype(BF16)
            return out.reshape(E_PAD * 72)

        s0 = pad72(np.ascontiguousarray(sp[..., 0]))          # (g, k)
        s1 = pad72(np.ascontiguousarray(sp[..., 1]))
        a = pad72(sp.sum(axis=3))

        en = elem_nodes[sl]                                   # [E, 8]
        disp = inputs[:, en]                                  # [B, E, 8, 2]

        # displacement planes: per chunk, (p, b, c, k) blocks, flat
        def dplane(x):                                        # x: [B, E, 8] f32
            xp = np.zeros((B, E_PAD, 8), np.float32)
            xp[:, :E_CORE] = x
            parts = []
            for e0, c in SEGS:
                blk = xp[:, e0:e0 + P * c]                    # [B, P*c, 8]
                blk = blk.reshape(B, P, c, 8).transpose(1, 0, 2, 3)
                parts.append(np.ascontiguousarray(blk).reshape(-1))
            return np.concatenate(parts).astype(BF16)

        u = disp[..., 0]
        v = disp[..., 1]
        in_maps.append({
            "s0": s0, "s1": s1, "a": a,
            "du": dplane(u), "dv": dplane(v), "dw": dplane(u + v),
        })

    core_ids = list(range(N_CORES))
    res = run_bass_kernel_spmd(nc, in_maps, core_ids, trace=_want_trace)

    outs = []
    for core in range(N_CORES):
        o = res.results[core]["out"].reshape(B, E_PAD * 9, 3) # bf16
        outs.append(o[:, :E_CORE * 9, :].astype(np.float32))
    full = np.concatenate(outs, axis=1)                       # [B, N_ELEM*9, 3]
    if _want_trace:
        return full, res
    return full
